# revision 16
# baseline (speedup 1.0000x reference)
import sys

if "/opt/trn_rl_repo" not in sys.path:
    sys.path.insert(0, "/opt/trn_rl_repo")

import numpy as np

N = 3_000_000
NCORES = 8
NPC = N // NCORES          # 375_000 samples per core
PART = 128                 # SBUF partitions
SPP = 2944                 # samples per partition (padded)
NPADPC = PART * SPP        # 376_832
ROW = SPP * 9              # elements per partition
NT = 4                     # tiles per core
K = SPP // NT              # 736 samples per tile per partition

# Cofactor-form kernel.  With Z = cof(F) (F @ Z.T = det(F) I and
# adj(F^T F) = Z^T Z), the Piola stress P = dW/dF collapses to
#   P = th'_c f_rc + sum_m w''_rm z_mc,
#   W'' = beta (Z G Z^T) + alpha I,  th'_c = 16 + 0.8 I4 g_c,
#   I4 = sum_rc g_c f_rc^2,  I5 = tr(Z G Z^T),  J = det F,
#   alpha = 20 J + (0.8 I5^2 - 56)/J,  beta = -0.8 I5 / J.
# All plane sums ride the PE as identity-weight matmul accumulations into
# PSUM: the G weights are folded into ACT Square scales (SF' = g_c f^2,
# SZ' = g_j z^2 via a sqrt(g)-column-scaled copy of Z), so every matmul
# uses the SAME stationary identity (no weight switches; codegen still
# pairs an LDWEIGHTS per MM, but identical loads pipeline).  1/J comes
# from the one-instruction DVE reciprocal_approx_fast (no Ln/Exp ACT
# table loads).  4 tiles, software-pipelined: tile t+1's Z-build + det
# chain fill the DVE stall while tile t's scalar drains complete; Q5/I5M
# (the gates of the per-tile DVE scalar chain) drain PSUM first, WO/I4
# after, with a dedicated PSUM tag for I5 to avoid a drain-order cycle.

_cache = {}

SQRT08 = 0.8944271909999159   # sqrt(0.8)
SQRTH = 0.7071067811865476    # sqrt(0.5)


class _Tile:
    pass


def _build():
    import concourse.bass as bass
    import concourse.tile as tile
    from concourse import bacc, mybir
    from contextlib import ExitStack

    f16 = mybir.dt.float16
    f32 = mybir.dt.float32
    AF = mybir.ActivationFunctionType

    nc = bacc.Bacc("TRN2", target_bir_lowering=False, debug=False)
    fin_d = nc.dram_tensor("fin", [PART, ROW], f16, kind="ExternalInput").ap()
    eye_d = nc.dram_tensor("eye", [PART, PART], f16, kind="ExternalInput").ap()
    pout_d = nc.dram_tensor("pout", [PART, ROW], f16, kind="ExternalOutput").ap()

    AL = mybir.AluOpType
    TT = nc.vector.tensor_tensor
    STT = nc.vector.scalar_tensor_tensor
    ACT = nc.scalar.activation
    P = PART

    with tile.TileContext(nc) as tc:
        with ExitStack() as ctx:
            io = ctx.enter_context(tc.tile_pool(name="io", bufs=2))
            sp = ctx.enter_context(tc.tile_pool(name="sp", bufs=1))
            ps = ctx.enter_context(
                tc.tile_pool(name="ps", bufs=1, space=bass.MemorySpace.PSUM))
            eye = sp.tile([P, P], f16, name="EYE", tag="EYE")
            nc.sync.dma_start(eye, eye_d)
            nc.tensor.ldweights(eye)          # the one and only weight load

            _ptag = [0]

            def _mm(out, rhs, start, stop):
                """matmul vs the resident identity weights (no reload)."""
                te = nc.tensor
                ifmap_ap = te.lower_ap(rhs.opt({0}), opt=False)
                weights_ap = te.lower_ap(
                    eye.opt({0}), opt=False, for_matmul_weights=True)
                out_ap = te.lower_ap(out)
                return te.add_instruction(
                    mybir.InstMatmult(
                        name=nc.get_next_instruction_name(),
                        replication_resolution=0,
                        replication_shift_amnt=0,
                        replication_num_rows=0,
                        start_tensor_calc=start,
                        stop_tensor_calc=stop,
                        ins=[ifmap_ap, weights_ap],
                        outs=[out_ap],
                        ldweights=False,
                        tile_position=(0, 0),
                        tile_size=(P, P),
                    ))

            def acc_psum(planes, tag=None):
                """Sum fp16 planes into a PSUM fp32 accumulator on the PE."""
                if tag is None:
                    _ptag[0] = (_ptag[0] + 1) % 3
                    tag = f"ps{_ptag[0]}"
                acc = ps.tile([P, K], f32, name=tag, tag=tag)
                flat = [(pl.rearrange("p n k -> p (n k)")
                         if len(pl.shape) == 3 else pl) for pl in planes]
                for i, pl in enumerate(flat):
                    first, last = (i == 0), (i == len(flat) - 1)
                    for lo in range(0, K, 512):
                        hi = min(lo + 512, K)
                        _mm(acc[:, lo:hi], pl[:, lo:hi], first, last)
                return acc

            def tl(tag, n, pool=None, bufs=1):
                pl = pool or sp
                return pl.tile([P, n, K], f16, name=tag, tag=tag, bufs=bufs)

            def bc(plane, n):
                return plane.broadcast_to([P, n, K])

            def fl(x):
                return x.rearrange("p n k -> p (n k)")

            tiles = [_Tile() for _ in range(NT)]

            def st_dma(t):
                s = tiles[t]
                s.inv = fin_d.rearrange("p (n s) -> p n s", n=9, s=SPP)[
                    :, :, t * K:(t + 1) * K]
                s.outv = pout_d.rearrange("p (n s) -> p n s", n=9, s=SPP)[
                    :, :, t * K:(t + 1) * K]
                s.FP = tl("FP", 9, pool=io, bufs=2)
                for r in range(3):
                    nc.sync.dma_start(s.FP[:, 3 * r:3 * r + 3, :],
                                      s.inv[:, 3 * r:3 * r + 3, :])

            def st_sq(t):
                # SF'_{3r+c} = g_c f_rc^2  (G folded into the Square scale)
                s = tiles[t]
                s.SF = tl("SF", 9)
                ACT(s.SF[:, 0::3, :], s.FP[:, 0::3, :], AF.Square, scale=2.0)
                ACT(s.SF[:, 1::3, :], s.FP[:, 1::3, :], AF.Square, scale=SQRTH)
                ACT(s.SF[:, 2::3, :], s.FP[:, 2::3, :], AF.Square, scale=SQRTH)

            def st_z(t):
                s = tiles[t]
                s.Z = tl(f"Z{t % 2}", 9)
                for r in (2, 0, 1):
                    r1, r2 = (r + 1) % 3, (r + 2) % 3
                    a, b = 3 * r1, 3 * r2
                    PA = tl("PA", 3)
                    TT(PA[:, 0:2, :], s.FP[:, a + 1:a + 3, :],
                       s.FP[:, b + 2:(b - 1 if b else None):-2, :], AL.mult)
                    TT(PA[:, 2:3, :], s.FP[:, a:a + 1, :],
                       s.FP[:, b + 1:b + 2, :], AL.mult)
                    PB = tl("PB", 3)
                    TT(PB[:, 0:2, :], s.FP[:, a + 2:(a - 1 if a else None):-2, :],
                       s.FP[:, b + 1:b + 3, :], AL.mult)
                    TT(PB[:, 2:3, :], s.FP[:, a + 1:a + 2, :],
                       s.FP[:, b:b + 1, :], AL.mult)
                    TT(s.Z[:, 3 * r:3 * r + 3, :], PA, PB, AL.subtract)

            def st_jchain(t):
                s = tiles[t]
                JP = tl("JP", 3)
                TT(JP, s.FP[:, 0:3, :], s.Z[:, 0:3, :], AL.mult)
                jacc = acc_psum([JP[:, c:c + 1, :] for c in range(3)])
                s.JS = tl(f"JS{t % 2}", 1)
                ACT(fl(s.JS), jacc, AF.Copy)
                s.RJ = sp.tile([P, 1, K], f32, name=f"RJ{t % 2}",
                               tag=f"RJ{t % 2}")
                nc.vector.reciprocal_approx_fast(fl(s.RJ), jacc)

            def st_mid(t):
                s = tiles[t]
                ZG = tl("ZG", 9)   # sqrt(g)-column-scaled Z
                nc.vector.tensor_scalar_mul(ZG[:, 0::3, :], s.Z[:, 0::3, :], 2.0)
                nc.vector.tensor_scalar_mul(ZG[:, 1::3, :], s.Z[:, 1::3, :], SQRTH)
                nc.vector.tensor_scalar_mul(ZG[:, 2::3, :], s.Z[:, 2::3, :], SQRTH)
                SZ = tl("SZ", 9)
                ACT(SZ, ZG, AF.Square)
                Q = tl("Q", 9)
                TT(Q[:, 0:3, :], ZG[:, 0:3, :], ZG[:, 3:6, :], AL.mult)
                TT(Q[:, 3:6, :], ZG[:, 0:3, :], ZG[:, 6:9, :], AL.mult)
                TT(Q[:, 6:9, :], ZG[:, 3:6, :], ZG[:, 6:9, :], AL.mult)
                # WD accs + drains first: they gate i5acc -> Q5/I5M, which
                # gate the tile's DVE scalar chain. WO/I4 drains come after.
                s.WD = tl("WD", 3)
                for r in range(3):
                    acc = acc_psum([SZ[:, 3 * r + j:3 * r + j + 1, :]
                                    for j in range(3)])
                    ACT(fl(s.WD[:, r:r + 1, :]), acc, AF.Copy)
                wo_accs = []
                for i in range(3):
                    wo_accs.append(acc_psum([Q[:, 3 * i + j:3 * i + j + 1, :]
                                             for j in range(3)]))
                i5acc = acc_psum([s.WD[:, r:r + 1, :] for r in range(3)],
                                 tag="psI5")
                s.Q5 = tl("Q5", 1)
                ACT(fl(s.Q5), i5acc, AF.Square, scale=SQRT08)
                s.I5M = tl("I5M", 1)
                ACT(fl(s.I5M), i5acc, AF.Copy, scale=-0.8)
                s.WO = tl("WO", 3)
                for i in range(3):
                    ACT(fl(s.WO[:, i:i + 1, :]), wo_accs[i], AF.Copy)
                i4acc = acc_psum([s.SF[:, i:i + 1, :] for i in range(9)])
                s.TP = tl("TP", 3)   # th'_c = 16 + 0.8 g_c I4
                ACT(fl(s.TP[:, 0:1, :]), i4acc, AF.Copy, scale=3.2, bias=16.0)
                ACT(fl(s.TP[:, 1:2, :]), i4acc, AF.Copy, scale=0.4, bias=16.0)
                ACT(fl(s.TP[:, 2:3, :]), i4acc, AF.Copy, scale=0.4, bias=16.0)

            def st_scal(t):
                s = tiles[t]
                TD = tl("TD", 1)
                STT(TD, s.Q5, -56.0, s.RJ, AL.add, AL.mult)
                UD = tl("UD", 1)
                nc.vector.tensor_scalar_mul(UD, s.JS, 20.0)
                ALP = tl("ALP", 1)
                TT(ALP, TD, UD, AL.add)
                BET = tl("BET", 1)
                TT(BET, s.I5M, s.RJ, AL.mult)
                s.WPO = tl("WPO", 3)
                TT(s.WPO, bc(BET, 3), s.WO, AL.mult)
                WPD = tl("PA", 3)
                TT(WPD, bc(BET, 3), s.WD, AL.mult)
                s.WDD = tl("WDD", 3)
                TT(s.WDD, WPD, bc(ALP, 3), AL.add)
                s.TF = tl("TF", 9)   # th'_c * f_rc
                for r in range(3):
                    TT(s.TF[:, 3 * r:3 * r + 3, :], s.TP,
                       s.FP[:, 3 * r:3 * r + 3, :], AL.mult)

            def st_waves(t):
                s = tiles[t]
                wrow = [[(s.WDD, 0), (s.WPO, 0), (s.WPO, 1)],
                        [(s.WPO, 0), (s.WDD, 1), (s.WPO, 2)],
                        [(s.WPO, 1), (s.WPO, 2), (s.WDD, 2)]]
                for r in range(3):
                    V = tl(f"V{r % 2}", 9)
                    for m in range(3):
                        wt, mi = wrow[r][m]
                        TT(V[:, 3 * m:3 * m + 3, :],
                           bc(wt[:, mi:mi + 1, :], 3),
                           s.Z[:, 3 * m:3 * m + 3, :], AL.mult)
                    PF = tl("PF", 3, pool=io, bufs=1)
                    for c in range(3):
                        acc = acc_psum([
                            s.TF[:, 3 * r + c:3 * r + c + 1, :],
                            V[:, c:c + 1, :],
                            V[:, 3 + c:4 + c, :],
                            V[:, 6 + c:7 + c, :],
                        ])
                        ACT(fl(PF[:, c:c + 1, :]), acc, AF.Copy)
                    nc.sync.dma_start(s.outv[:, 3 * r:3 * r + 3, :], PF)

            # ---- pipeline schedule ----
            st_dma(0)
            st_sq(0)
            st_dma(1)
            st_z(0)
            st_jchain(0)
            st_mid(0)
            for t in range(NT):
                if t + 1 < NT:
                    # filler for tile t's scalar-drain latency
                    st_z(t + 1)
                    st_jchain(t + 1)
                    st_sq(t + 1)
                if t + 2 < NT:
                    st_dma(t + 2)
                st_scal(t)
                st_waves(t)
                if t + 1 < NT:
                    st_mid(t + 1)

    nc.compile()
    return nc


def _get_nc():
    if "nc" not in _cache:
        _cache["nc"] = _build()
    return _cache["nc"]


def _make_in_maps(F):
    x = F.reshape(N, 9).astype(np.float16)
    eye9 = np.array([1, 0, 0, 0, 1, 0, 0, 0, 1], dtype=np.float16)
    pad = np.tile(eye9, (NPADPC - NPC, 1))
    eye = np.eye(PART, dtype=np.float16)
    in_maps = []
    for cidx in range(NCORES):
        xc = x[cidx * NPC:(cidx + 1) * NPC]
        xcp = (np.concatenate([xc, pad], axis=0)
               .reshape(PART, SPP, 9).transpose(0, 2, 1).reshape(PART, ROW))
        in_maps.append({"fin": np.ascontiguousarray(xcp), "eye": eye})
    return in_maps


def kernel(**inputs):
    from concourse.bass_utils import run_bass_kernel_spmd

    F = np.asarray(inputs["F"], dtype=np.float32)
    nc = _get_nc()
    in_maps = _make_in_maps(F)

    res = run_bass_kernel_spmd(nc, in_maps, list(range(NCORES)))

    out = np.empty((N, 9), dtype=np.float32)
    for cidx in range(NCORES):
        oc = (np.asarray(res.results[cidx]["pout"]).astype(np.float32)
              .reshape(PART, 9, SPP).transpose(0, 2, 1).reshape(NPADPC, 9))
        out[cidx * NPC:(cidx + 1) * NPC] = oc[:NPC]
    return out.reshape(N, 3, 3)


# revision 18
# speedup vs baseline: 1.0168x; 1.0168x over previous
import sys

if "/opt/trn_rl_repo" not in sys.path:
    sys.path.insert(0, "/opt/trn_rl_repo")

import numpy as np

N = 3_000_000
NCORES = 8
NPC = N // NCORES          # 375_000 samples per core
PART = 128                 # SBUF partitions
SPP = 2944                 # samples per partition (padded)
NPADPC = PART * SPP        # 376_832
ROW = SPP * 9              # elements per partition
NT = 4                     # tiles per core
K = SPP // NT              # 736 samples per tile per partition

# Cofactor-form kernel.  With Z = cof(F) (F @ Z.T = det(F) I and
# adj(F^T F) = Z^T Z), the Piola stress P = dW/dF collapses to
#   P = th'_c f_rc + sum_m w''_rm z_mc,
#   W'' = beta (Z G Z^T) + alpha I,  th'_c = 16 + 0.8 I4 g_c,
#   I4 = sum_rc g_c f_rc^2,  I5 = tr(Z G Z^T),  J = det F,
#   alpha = 20 J + (0.8 I5^2 - 56)/J,  beta = -0.8 I5 / J.
# All plane sums ride the PE as identity-weight matmul accumulations into
# PSUM: the G weights fold into ACT Square scales (SF' = g_c f^2,
# SZ' = g_j z^2) or into the stationary diag weights (Wo).  1/J comes
# from the one-instruction DVE reciprocal_approx_fast (no Ln/Exp ACT
# table loads).  4 tiles, software-pipelined: tile t+1's Z-build + det
# chain fill the DVE stall while tile t's scalar drains complete; Q5/I5M
# (the gates of the per-tile DVE scalar chain) drain PSUM first, WO/I4
# after, with a dedicated PSUM tag for I5 to avoid a drain-order cycle.

_cache = {}

SQRT08 = 0.8944271909999159   # sqrt(0.8)
SQRTH = 0.7071067811865476    # sqrt(0.5)


class _Tile:
    pass


def _build():
    import concourse.bass as bass
    import concourse.tile as tile
    from concourse import bacc, mybir
    from contextlib import ExitStack

    f16 = mybir.dt.float16
    f32 = mybir.dt.float32
    AF = mybir.ActivationFunctionType

    nc = bacc.Bacc("TRN2", target_bir_lowering=False, debug=False)
    fin_d = nc.dram_tensor("fin", [PART, ROW], f16, kind="ExternalInput").ap()
    eye_d = nc.dram_tensor("eye", [3, PART, PART], f16, kind="ExternalInput").ap()
    pout_d = nc.dram_tensor("pout", [PART, ROW], f16, kind="ExternalOutput").ap()

    AL = mybir.AluOpType
    TT = nc.vector.tensor_tensor
    STT = nc.vector.scalar_tensor_tensor
    ACT = nc.scalar.activation
    P = PART

    with tile.TileContext(nc) as tc:
        with ExitStack() as ctx:
            io = ctx.enter_context(tc.tile_pool(name="io", bufs=2))
            sp = ctx.enter_context(tc.tile_pool(name="sp", bufs=1))
            ps = ctx.enter_context(
                tc.tile_pool(name="ps", bufs=1, space=bass.MemorySpace.PSUM))
            eyes = sp.tile([P, 3, P], f16, name="EYE", tag="EYE")
            nc.sync.dma_start(eyes, eye_d.transpose([1, 0, 2]))
            EYE1, EYE4, EYEH = (eyes[:, i, :] for i in range(3))
            GW = (EYE4, EYEH, EYEH)
            nc.tensor.ldweights(EYE1)

            _ptag = [0]

            def _mm(out, rhs, w, start, stop):
                te = nc.tensor
                ifmap_ap = te.lower_ap(rhs.opt({0}), opt=False)
                weights_ap = te.lower_ap(
                    w.opt({0}), opt=False, for_matmul_weights=True)
                out_ap = te.lower_ap(out)
                return te.add_instruction(
                    mybir.InstMatmult(
                        name=nc.get_next_instruction_name(),
                        replication_resolution=0,
                        replication_shift_amnt=0,
                        replication_num_rows=0,
                        start_tensor_calc=start,
                        stop_tensor_calc=stop,
                        ins=[ifmap_ap, weights_ap],
                        outs=[out_ap],
                        ldweights=False,
                        tile_position=(0, 0),
                        tile_size=(P, P),
                    ))

            def acc_psum(planes, tag=None):
                """Sum fp16 planes into a PSUM fp32 accumulator on the PE."""
                if tag is None:
                    _ptag[0] = (_ptag[0] + 1) % 3
                    tag = f"ps{_ptag[0]}"
                acc = ps.tile([P, K], f32, name=tag, tag=tag)
                flat = []
                for pl in planes:
                    pl, w = pl if isinstance(pl, tuple) else (pl, EYE1)
                    if len(pl.shape) == 3:
                        pl = pl.rearrange("p n k -> p (n k)")
                    flat.append((pl, w))
                for i, (pl, w) in enumerate(flat):
                    first, last = (i == 0), (i == len(flat) - 1)
                    for lo in range(0, K, 512):
                        hi = min(lo + 512, K)
                        _mm(acc[:, lo:hi], pl[:, lo:hi], w, first, last)
                return acc

            def tl(tag, n, pool=None, bufs=1):
                pl = pool or sp
                return pl.tile([P, n, K], f16, name=tag, tag=tag, bufs=bufs)

            def bc(plane, n):
                return plane.broadcast_to([P, n, K])

            def fl(x):
                return x.rearrange("p n k -> p (n k)")

            tiles = [_Tile() for _ in range(NT)]

            def st_dma(t):
                s = tiles[t]
                s.inv = fin_d.rearrange("p (n s) -> p n s", n=9, s=SPP)[
                    :, :, t * K:(t + 1) * K]
                s.outv = pout_d.rearrange("p (n s) -> p n s", n=9, s=SPP)[
                    :, :, t * K:(t + 1) * K]
                s.FP = tl("FP", 9, pool=io, bufs=2)
                for r in range(3):
                    nc.sync.dma_start(s.FP[:, 3 * r:3 * r + 3, :],
                                      s.inv[:, 3 * r:3 * r + 3, :])

            def st_sq(t):
                # SF'_{3r+c} = g_c f_rc^2  (G folded into the Square scale)
                s = tiles[t]
                s.SF = tl("SF", 9)
                ACT(s.SF[:, 0::3, :], s.FP[:, 0::3, :], AF.Square, scale=2.0)
                ACT(s.SF[:, 1::3, :], s.FP[:, 1::3, :], AF.Square, scale=SQRTH)
                ACT(s.SF[:, 2::3, :], s.FP[:, 2::3, :], AF.Square, scale=SQRTH)

            def st_z(t):
                s = tiles[t]
                s.Z = tl(f"Z{t % 2}", 9)
                for r in (2, 0, 1):
                    r1, r2 = (r + 1) % 3, (r + 2) % 3
                    a, b = 3 * r1, 3 * r2
                    PA = tl("PA", 3)
                    TT(PA[:, 0:2, :], s.FP[:, a + 1:a + 3, :],
                       s.FP[:, b + 2:(b - 1 if b else None):-2, :], AL.mult)
                    TT(PA[:, 2:3, :], s.FP[:, a:a + 1, :],
                       s.FP[:, b + 1:b + 2, :], AL.mult)
                    PB = tl("PB", 3)
                    TT(PB[:, 0:2, :], s.FP[:, a + 2:(a - 1 if a else None):-2, :],
                       s.FP[:, b + 1:b + 3, :], AL.mult)
                    TT(PB[:, 2:3, :], s.FP[:, a + 1:a + 2, :],
                       s.FP[:, b:b + 1, :], AL.mult)
                    TT(s.Z[:, 3 * r:3 * r + 3, :], PA, PB, AL.subtract)

            def st_jchain(t):
                s = tiles[t]
                JP = tl("JP", 3)
                TT(JP, s.FP[:, 0:3, :], s.Z[:, 0:3, :], AL.mult)
                jacc = acc_psum([JP[:, c:c + 1, :] for c in range(3)])
                s.JS = tl(f"JS{t % 2}", 1)
                ACT(fl(s.JS), jacc, AF.Copy)
                s.RJ = sp.tile([P, 1, K], f32, name=f"RJ{t % 2}",
                               tag=f"RJ{t % 2}")
                nc.vector.reciprocal_approx_fast(fl(s.RJ), jacc)

            def st_mid(t):
                s = tiles[t]
                SZ = tl("SZ", 9)   # g_j z_rj^2 (g folded into Square scale)
                ACT(SZ[:, 0::3, :], s.Z[:, 0::3, :], AF.Square, scale=2.0)
                ACT(SZ[:, 1::3, :], s.Z[:, 1::3, :], AF.Square, scale=SQRTH)
                ACT(SZ[:, 2::3, :], s.Z[:, 2::3, :], AF.Square, scale=SQRTH)
                Q = tl("Q", 9)
                TT(Q[:, 0:3, :], s.Z[:, 0:3, :], s.Z[:, 3:6, :], AL.mult)
                TT(Q[:, 3:6, :], s.Z[:, 0:3, :], s.Z[:, 6:9, :], AL.mult)
                TT(Q[:, 6:9, :], s.Z[:, 3:6, :], s.Z[:, 6:9, :], AL.mult)
                # WD accs + drains first: they gate i5acc -> Q5/I5M, which
                # gate the tile's DVE scalar chain. WO/I4 drains come after.
                s.WD = tl("WD", 3)
                for r in range(3):
                    acc = acc_psum([SZ[:, 3 * r + j:3 * r + j + 1, :]
                                    for j in range(3)])
                    ACT(fl(s.WD[:, r:r + 1, :]), acc, AF.Copy)
                wo_accs = []
                for i in range(3):
                    wo_accs.append(acc_psum(
                        [(Q[:, 3 * i + j:3 * i + j + 1, :], GW[j])
                         for j in range(3)]))
                i5acc = acc_psum([s.WD[:, r:r + 1, :] for r in range(3)],
                                 tag="psI5")
                s.Q5 = tl("Q5", 1)
                ACT(fl(s.Q5), i5acc, AF.Square, scale=SQRT08)
                s.I5M = tl("I5M", 1)
                ACT(fl(s.I5M), i5acc, AF.Copy, scale=-0.8)
                s.WO = tl("WO", 3)
                for i in range(3):
                    ACT(fl(s.WO[:, i:i + 1, :]), wo_accs[i], AF.Copy)
                i4acc = acc_psum([s.SF[:, i:i + 1, :] for i in range(9)])
                s.TP = tl("TP", 3)   # th'_c = 16 + 0.8 g_c I4
                ACT(fl(s.TP[:, 0:1, :]), i4acc, AF.Copy, scale=3.2, bias=16.0)
                ACT(fl(s.TP[:, 1:2, :]), i4acc, AF.Copy, scale=0.4, bias=16.0)
                ACT(fl(s.TP[:, 2:3, :]), i4acc, AF.Copy, scale=0.4, bias=16.0)

            def st_scal(t):
                s = tiles[t]
                TD = tl("TD", 1)
                STT(TD, s.Q5, -56.0, s.RJ, AL.add, AL.mult)
                UD = tl("UD", 1)
                nc.vector.tensor_scalar_mul(UD, s.JS, 20.0)
                ALP = tl("ALP", 1)
                TT(ALP, TD, UD, AL.add)
                BET = tl("BET", 1)
                TT(BET, s.I5M, s.RJ, AL.mult)
                s.WPO = tl("WPO", 3)
                TT(s.WPO, bc(BET, 3), s.WO, AL.mult)
                WPD = tl("PA", 3)
                TT(WPD, bc(BET, 3), s.WD, AL.mult)
                s.WDD = tl("WDD", 3)
                TT(s.WDD, WPD, bc(ALP, 3), AL.add)
                s.TF = tl("TF", 9)   # th'_c * f_rc
                for r in range(3):
                    TT(s.TF[:, 3 * r:3 * r + 3, :], s.TP,
                       s.FP[:, 3 * r:3 * r + 3, :], AL.mult)

            def st_waves(t):
                s = tiles[t]
                wrow = [[(s.WDD, 0), (s.WPO, 0), (s.WPO, 1)],
                        [(s.WPO, 0), (s.WDD, 1), (s.WPO, 2)],
                        [(s.WPO, 1), (s.WPO, 2), (s.WDD, 2)]]
                for r in range(3):
                    V = tl(f"V{r % 2}", 9)
                    for m in range(3):
                        wt, mi = wrow[r][m]
                        TT(V[:, 3 * m:3 * m + 3, :],
                           bc(wt[:, mi:mi + 1, :], 3),
                           s.Z[:, 3 * m:3 * m + 3, :], AL.mult)
                    PF = tl("PF", 3, pool=io, bufs=1)
                    for c in range(3):
                        acc = acc_psum([
                            s.TF[:, 3 * r + c:3 * r + c + 1, :],
                            V[:, c:c + 1, :],
                            V[:, 3 + c:4 + c, :],
                            V[:, 6 + c:7 + c, :],
                        ])
                        ACT(fl(PF[:, c:c + 1, :]), acc, AF.Copy)
                    nc.sync.dma_start(s.outv[:, 3 * r:3 * r + 3, :], PF)

            # ---- pipeline schedule ----
            st_dma(0)
            st_sq(0)
            st_dma(1)
            st_z(0)
            st_jchain(0)
            st_mid(0)
            for t in range(NT):
                if t + 1 < NT:
                    # filler for tile t's scalar-drain latency
                    st_z(t + 1)
                    st_jchain(t + 1)
                    st_sq(t + 1)
                if t + 2 < NT:
                    st_dma(t + 2)
                st_scal(t)
                st_waves(t)
                if t + 1 < NT:
                    st_mid(t + 1)

    nc.compile()
    return nc


def _get_nc():
    if "nc" not in _cache:
        _cache["nc"] = _build()
    return _cache["nc"]


def _make_in_maps(F):
    x = F.reshape(N, 9).astype(np.float16)
    eye9 = np.array([1, 0, 0, 0, 1, 0, 0, 0, 1], dtype=np.float16)
    pad = np.tile(eye9, (NPADPC - NPC, 1))
    e = np.eye(PART, dtype=np.float16)
    eye = np.stack([e, 4.0 * e, 0.5 * e])
    in_maps = []
    for cidx in range(NCORES):
        xc = x[cidx * NPC:(cidx + 1) * NPC]
        xcp = (np.concatenate([xc, pad], axis=0)
               .reshape(PART, SPP, 9).transpose(0, 2, 1).reshape(PART, ROW))
        in_maps.append({"fin": np.ascontiguousarray(xcp), "eye": eye})
    return in_maps


def kernel(**inputs):
    from concourse.bass_utils import run_bass_kernel_spmd

    F = np.asarray(inputs["F"], dtype=np.float32)
    nc = _get_nc()
    in_maps = _make_in_maps(F)

    res = run_bass_kernel_spmd(nc, in_maps, list(range(NCORES)))

    out = np.empty((N, 9), dtype=np.float32)
    for cidx in range(NCORES):
        oc = (np.asarray(res.results[cidx]["pout"]).astype(np.float32)
              .reshape(PART, 9, SPP).transpose(0, 2, 1).reshape(NPADPC, 9))
        out[cidx * NPC:(cidx + 1) * NPC] = oc[:NPC]
    return out.reshape(N, 3, 3)
